# revision 2
# baseline (speedup 1.0000x reference)
"""DeltaResidualBlock (dense_mlp) Trainium2 Bass/Tile kernel.

Contract: kernel(**inputs) takes the FULL unsharded inputs of
nn_DeltaResidualBlock_11063835755016:
    h  (4, 4096, 2048) f32, Wk (2048, 2048) f32, Wb (2048,) f32,
    bb (1,) f32, Wv (2048,) f32, bv (1,) f32
and returns (h_next (4, 4096, 2048) f32, beta_mean () f32) matching:

    k_raw  = h @ Wk^T
    k      = k_raw / (||k_raw||_2 + 1e-8)
    beta   = 2 sigmoid(h @ Wb + bb)
    v      = h @ Wv + bv
    h_next = h + beta * k * (v - sum(k*h))
    beta_mean = mean(beta)

Sharding: data-parallel over the 16384 tokens across 8 NeuronCores
(2048 tokens per core); projection weights replicated (Wk shipped
pre-transposed — a host-side layout choice only).  Each core returns its
h_next shard plus a [128,1] vector of per-partition beta partial sums;
the host concatenates shards and finishes the beta mean.

Per-core pipeline, per 128-token tile (software-pipelined `pf` deep):
    DMA h tile (f32) -> ACT cast to bf16 -> xbar DMA-transpose (hT)
    -> PE matmul (d-contracted, k_raw [128,2048] accumulated in PSUM)
    -> ACT square+accum (ssq) / DVE mul+reduce (dot) / GpSimd muls +
    DVE reduces (v, blogit) -> per-token scalar chain -> fused DVE
    (k_raw*alpha)+h -> DMA out.
The 16 Wk slices load+cast interleaved into the prologue prefetches so
the PE does not wait on the full 16.8 MB weight load.
"""

from contextlib import ExitStack

import numpy as np

import concourse.bass as bass
import concourse.mybir as mybir
import concourse.tile as tile
from concourse import bacc
from concourse.bass_utils import run_bass_kernel_spmd

DIM = 2048
EPS = 1e-8
P = 128
N_CHUNK = 512
N_CORES = 8
TOKENS_PER_CORE = 4 * 4096 // N_CORES
DTYPE = "bf16"  # "bf16" | "fp8"


def build(tokens: int = TOKENS_PER_CORE, reps: int = 1, n_cores: int = N_CORES,
          dtype: str = None, pf: int = 3, vb_eng: str = "gpsimd",
          wk_scale: float = 64.0):
    dtype = dtype or DTYPE
    assert tokens % P == 0
    n_tiles = tokens // P
    d_tiles = DIM // P
    e_chunks = DIM // N_CHUNK
    fp32 = mybir.dt.float32
    bf16 = mybir.dt.bfloat16
    fp8 = mybir.dt.float8e4
    dr = dtype == "fp8"
    mm_dt = fp8 if dr else bf16
    perf_mode = mybir.MatmulPerfMode.DoubleRow if dr else None
    kscale = wk_scale if dr else 1.0

    nc = bacc.Bacc("TRN2", target_bir_lowering=False, debug=False,
                   num_devices=n_cores)
    h_d = nc.dram_tensor("h", [tokens, DIM], fp32, kind="ExternalInput")
    wkT_d = nc.dram_tensor("wkT", [DIM, DIM], fp32, kind="ExternalInput")
    wv_d = nc.dram_tensor("wv", [DIM], fp32, kind="ExternalInput")
    wb_d = nc.dram_tensor("wb", [DIM], fp32, kind="ExternalInput")
    bb_d = nc.dram_tensor("bb", [1], fp32, kind="ExternalInput")
    bv_d = nc.dram_tensor("bv", [1], fp32, kind="ExternalInput")
    out_d = nc.dram_tensor("out", [tokens, DIM], fp32, kind="ExternalOutput")
    bp_d = nc.dram_tensor("beta_part", [P, 1], fp32, kind="ExternalOutput")

    A = mybir.AluOpType
    AF = mybir.ActivationFunctionType

    def bcast(dram_ap, n):
        return bass.AP(tensor=dram_ap.tensor, offset=dram_ap.offset,
                       ap=[[0, P], [1, n]])

    with tile.TileContext(nc) as tc, ExitStack() as ctx:
        singles = ctx.enter_context(tc.tile_pool(name="singles", bufs=1))
        stg = ctx.enter_context(tc.tile_pool(name="stg", bufs=2))
        hp = ctx.enter_context(tc.tile_pool(name="hp", bufs=pf + 1))
        htp = ctx.enter_context(tc.tile_pool(name="htp", bufs=pf + 1))
        op = ctx.enter_context(tc.tile_pool(name="op", bufs=3))
        scr = ctx.enter_context(tc.tile_pool(name="scr", bufs=4))
        sm = ctx.enter_context(tc.tile_pool(name="sm", bufs=8))
        pp = ctx.enter_context(tc.tile_pool(name="pp", bufs=2, space="PSUM"))

        wkT_sb = singles.tile([P, d_tiles, DIM], mm_dt)

        def load_wk(j):
            stage = stg.tile([P, DIM], fp32, tag="wk_stage")
            nc.sync.dma_start(stage[:], wkT_d[j * P:(j + 1) * P, :])
            if kscale != 1.0:
                nc.scalar.activation(wkT_sb[:, j, :], stage[:], AF.Copy,
                                     bias=0.0, scale=kscale)
            else:
                nc.vector.tensor_copy(wkT_sb[:, j, :], stage[:])

        wv_rep = singles.tile([P, DIM], fp32)
        nc.sync.dma_start(wv_rep[:], bcast(wv_d.ap(), DIM))
        wb_rep = singles.tile([P, DIM], fp32)
        nc.sync.dma_start(wb_rep[:], bcast(wb_d.ap(), DIM))
        bb_sb = singles.tile([P, 1], fp32)
        nc.sync.dma_start(bb_sb[:], bcast(bb_d.ap(), 1))
        bv_sb = singles.tile([P, 1], fp32)
        nc.sync.dma_start(bv_sb[:], bcast(bv_d.ap(), 1))
        beta_acc = singles.tile([P, 1], fp32)
        nc.vector.memset(beta_acc[:], 0.0)

        eng_vb = nc.vector if vb_eng == "vector" else nc.gpsimd

        hbufs = {}

        def prefetch(it, rep):
            if it >= n_tiles or rep >= reps:
                return
            t0 = it * P
            h_f32 = hp.tile([P, DIM], fp32, tag="h")
            nc.sync.dma_start(h_f32[:], h_d[t0:t0 + P, :])
            h_bf = hp.tile([P, DIM], bf16, tag="hbf")
            nc.scalar.copy(h_bf[:], h_f32[:])
            hT = htp.tile([P, d_tiles, P], bf16, tag="hT")
            nc.sync.dma_start_transpose(hT[:], h_bf[:])
            if dr:
                hT8 = htp.tile([P, d_tiles, P], fp8, tag="hT8")
                nc.vector.tensor_copy(
                    hT8[:].rearrange("p a b -> p (a b)"),
                    hT[:].rearrange("p a b -> p (a b)"))
                hbufs[(rep, it)] = (h_f32, hT8)
            else:
                hbufs[(rep, it)] = (h_f32, hT)

        # prologue: prefetch pf tiles, interleaving the 16 wk slice loads
        wk_done = 0
        for it in range(pf):
            prefetch(it, 0)
            target = d_tiles * (it + 1) // pf
            while wk_done < target:
                load_wk(wk_done)
                wk_done += 1
        while wk_done < d_tiles:
            load_wk(wk_done)
            wk_done += 1

        for rep in range(reps):
            for it in range(n_tiles):
                t0 = it * P
                h_f32, lht = hbufs.pop((rep, it))

                kraw = pp.tile([P, DIM], fp32, tag="kraw")
                if dr:
                    for j in range(d_tiles // 2):
                        lhsT = lht[:, 2 * j:2 * j + 2, :]
                        for e in range(e_chunks):
                            nc.tensor.matmul(
                                kraw[:, e * N_CHUNK:(e + 1) * N_CHUNK], lhsT,
                                wkT_sb[:, 2 * j:2 * j + 2,
                                       e * N_CHUNK:(e + 1) * N_CHUNK],
                                start=(j == 0), stop=(j == d_tiles // 2 - 1),
                                perf_mode=perf_mode)
                else:
                    for j in range(d_tiles):
                        lhsT = lht[:, j, :]
                        for e in range(e_chunks):
                            nc.tensor.matmul(
                                kraw[:, e * N_CHUNK:(e + 1) * N_CHUNK], lhsT,
                                wkT_sb[:, j, e * N_CHUNK:(e + 1) * N_CHUNK],
                                start=(j == 0), stop=(j == d_tiles - 1))

                nrep, nit = rep, it + pf
                if nit >= n_tiles:
                    nrep, nit = rep + 1, nit - n_tiles
                prefetch(nit, nrep)

                # ssq = sum(kraw^2) on ACT
                ssq = sm.tile([P, 1], fp32, tag="ssq")
                junk2 = scr.tile([P, DIM], bf16, tag="scr")
                nc.scalar.activation(junk2[:], kraw[:], AF.Square,
                                     accum_out=ssq[:])

                # dot = sum(kraw * h) on DVE
                dot = sm.tile([P, 1], fp32, tag="dot")
                junk3 = scr.tile([P, DIM], bf16, tag="scr")
                nc.vector.tensor_mul(junk3[:], kraw[:], h_f32[:])
                nc.vector.tensor_reduce(dot[:], junk3[:],
                                        axis=mybir.AxisListType.X, op=A.add)

                # v_raw, blogit: muls on GpSimd (idle), reduces on DVE
                v_raw = sm.tile([P, 1], fp32, tag="vraw")
                junk0 = scr.tile([P, DIM], bf16, tag="scr")
                eng_vb.tensor_mul(junk0[:], h_f32[:], wv_rep[:])
                nc.vector.tensor_reduce(v_raw[:], junk0[:],
                                        axis=mybir.AxisListType.X, op=A.add)
                b_raw = sm.tile([P, 1], fp32, tag="braw")
                junk1 = scr.tile([P, DIM], bf16, tag="scr")
                eng_vb.tensor_mul(junk1[:], h_f32[:], wb_rep[:])
                nc.vector.tensor_reduce(b_raw[:], junk1[:],
                                        axis=mybir.AxisListType.X, op=A.add)

                # s' = 1/(sqrt(ssq') + kscale*eps); kraw carries kscale, which
                # cancels in s'*dot' and alpha'*kraw'.
                nrm = sm.tile([P, 1], fp32, tag="nrm")
                nc.scalar.activation(nrm[:], ssq[:], AF.Sqrt)
                nc.vector.tensor_scalar_add(nrm[:], nrm[:], EPS * kscale)
                s = sm.tile([P, 1], fp32, tag="s")
                nc.vector.reciprocal(s[:], nrm[:])

                beta = sm.tile([P, 1], fp32, tag="beta")
                nc.scalar.activation(beta[:], b_raw[:], AF.Sigmoid,
                                     bias=bb_sb[:], scale=1.0)
                # beta_acc += 2*sigmoid (reference beta carries the factor 2)
                nc.vector.scalar_tensor_tensor(
                    out=beta_acc[:], in0=beta[:], scalar=2.0, in1=beta_acc[:],
                    op0=A.mult, op1=A.add)

                t1 = sm.tile([P, 1], fp32, tag="t1")
                nc.vector.tensor_mul(t1[:], s[:], dot[:])
                t2 = sm.tile([P, 1], fp32, tag="t2")
                nc.vector.scalar_tensor_tensor(
                    out=t2[:], in0=v_raw[:], scalar=bv_sb[:], in1=t1[:],
                    op0=A.add, op1=A.subtract)
                t3 = sm.tile([P, 1], fp32, tag="t3")
                nc.vector.tensor_mul(t3[:], beta[:], s[:])
                alpha = sm.tile([P, 1], fp32, tag="alpha")
                nc.vector.scalar_tensor_tensor(
                    out=alpha[:], in0=t3[:], scalar=2.0, in1=t2[:],
                    op0=A.mult, op1=A.mult)

                # out = h + alpha * kraw (fused on DVE)
                h_out = op.tile([P, DIM], fp32, tag="hout")
                nc.vector.scalar_tensor_tensor(
                    out=h_out[:], in0=kraw[:], scalar=alpha[:], in1=h_f32[:],
                    op0=A.mult, op1=A.add)
                nc.sync.dma_start(out_d[t0:t0 + P, :], h_out[:])

        nc.sync.dma_start(bp_d[:], beta_acc[:])

    nc.compile()
    return nc


def make_in_maps(h, Wk, Wb, bb, Wv, bv, n_cores: int = N_CORES):
    toks = np.asarray(h, dtype=np.float32).reshape(-1, DIM)
    shards = np.split(toks, n_cores, axis=0)
    wkT = np.ascontiguousarray(np.asarray(Wk, dtype=np.float32).T)
    return [
        {"h": np.ascontiguousarray(s), "wkT": wkT,
         "wv": np.asarray(Wv, dtype=np.float32),
         "wb": np.asarray(Wb, dtype=np.float32),
         "bb": np.asarray(bb, dtype=np.float32),
         "bv": np.asarray(bv, dtype=np.float32)}
        for s in shards
    ]


_NC_CACHE = {}


def _get_nc(reps: int = 1, dtype: str = None):
    key = (reps, dtype or DTYPE)
    if key not in _NC_CACHE:
        _NC_CACHE[key] = build(TOKENS_PER_CORE, reps=reps, dtype=dtype)
    return _NC_CACHE[key]


def kernel(h, Wk, Wb, bb, Wv, bv):
    nc = _get_nc()
    in_maps = make_in_maps(h, Wk, Wb, bb, Wv, bv)
    res = run_bass_kernel_spmd(nc, in_maps, list(range(N_CORES)))
    h_next = np.concatenate([r["out"] for r in res.results], axis=0)
    h_next = h_next.reshape(4, 4096, DIM)
    beta_sum = np.sum([r["beta_part"].sum(dtype=np.float64)
                       for r in res.results])
    beta_mean = np.float32(beta_sum / (4 * 4096))
    return h_next, beta_mean


# revision 3
# speedup vs baseline: 5.0653x; 5.0653x over previous
"""DeltaResidualBlock (dense_mlp) Trainium2 Bass/Tile kernel.

kernel(**inputs) takes the FULL unsharded inputs of
nn_DeltaResidualBlock_11063835755016:
    h (4, 4096, 2048) f32, Wk (2048, 2048) f32, Wb (2048,) f32,
    bb (1,) f32, Wv (2048,) f32, bv (1,) f32
and returns (h_next (4, 4096, 2048) f32, beta_mean () f32) matching:

    k_raw  = h @ Wk^T
    k      = k_raw / (||k_raw||_2 + 1e-8)
    beta   = 2 sigmoid(h @ Wb + bb)
    v      = h @ Wv + bv
    h_next = h + beta * k * (v - sum(k*h))
    beta_mean = mean(beta)

Sharding: data-parallel over the 16384 tokens across 8 NeuronCores (2048
tokens per core); projection weights replicated (Wk shipped
pre-transposed -- a host-side layout choice).  Each core returns its
h_next shard plus per-partition beta partial sums; the host concatenates
shards and finishes the beta mean.

Per-core pipeline (16 token-tiles of 128, processed in groups of 2):
  DMA h group (f32) -> ACT bf16 cast -> one xbar DMA-transpose -> DVE
  fp8 cast -> PE DoubleRow fp8 matmuls (k_raw in PSUM, f32 accum; Wk
  pre-scaled x64 into fp8 range -- the scale cancels through the L2
  normalization) -> ACT square+accum (ssq), DVE mul+reduce (dot),
  GpSimd muls + DVE reduces (v, blogit), per-group scalar chain ->
  fused DVE (k_raw*alpha)+h -> DMA out.  Wk slice loads interleave into
  the prologue prefetches.

fp8 end-to-end error vs the fp32 reference: rel ~6e-5, absmax ~8e-4
(the residual structure keeps matmul quantization noise ~1e-4 of the
output scale).
"""

from contextlib import ExitStack

import numpy as np

import concourse.bass as bass
import concourse.mybir as mybir
import concourse.tile as tile
from concourse import bacc
from concourse.bass_utils import run_bass_kernel_spmd

N_CORES = 8
TOKENS_PER_CORE = 4 * 4096 // N_CORES
DTYPE = "fp8"  # "fp8" | "bf16"

DIM = 2048
EPS = 1e-8
P = 128
N_CHUNK = 512
G = 2  # tiles per group


def build(tokens: int = TOKENS_PER_CORE, reps: int = 1,
          n_cores: int = N_CORES, dtype: str = None,
          wk_scale: float = 64.0):
    dtype = dtype or DTYPE
    assert tokens % (P * G) == 0
    n_tiles = tokens // P
    n_groups = n_tiles // G
    d_tiles = DIM // P
    e_chunks = DIM // N_CHUNK
    fp32 = mybir.dt.float32
    bf16 = mybir.dt.bfloat16
    fp8 = mybir.dt.float8e4
    dr = dtype == "fp8"
    mm_dt = fp8 if dr else bf16
    perf_mode = mybir.MatmulPerfMode.DoubleRow if dr else None
    kscale = wk_scale if dr else 1.0

    nc = bacc.Bacc("TRN2", target_bir_lowering=False, debug=False,
                   num_devices=n_cores)
    h_d = nc.dram_tensor("h", [tokens, DIM], fp32, kind="ExternalInput")
    wkT_d = nc.dram_tensor("wkT", [DIM, DIM], fp32, kind="ExternalInput")
    wv_d = nc.dram_tensor("wv", [DIM], fp32, kind="ExternalInput")
    wb_d = nc.dram_tensor("wb", [DIM], fp32, kind="ExternalInput")
    bb_d = nc.dram_tensor("bb", [1], fp32, kind="ExternalInput")
    bv_d = nc.dram_tensor("bv", [1], fp32, kind="ExternalInput")
    out_d = nc.dram_tensor("out", [tokens, DIM], fp32, kind="ExternalOutput")
    bp_d = nc.dram_tensor("beta_part", [P, G], fp32, kind="ExternalOutput")

    # grouped views: h[(g G + l)*P + p, d]
    h_g = h_d.ap().rearrange("(g l p) d -> g p l d", l=G, p=P)
    out_g = out_d.ap().rearrange("(g l p) d -> g p l d", l=G, p=P)
    wk_g = wkT_d.ap().rearrange("(s l p) e -> s p l e", l=G, p=P)

    A = mybir.AluOpType
    AF = mybir.ActivationFunctionType
    X = mybir.AxisListType.X

    def bcast(dram_ap, n):
        return bass.AP(tensor=dram_ap.tensor, offset=dram_ap.offset,
                       ap=[[0, P], [1, n]])

    with tile.TileContext(nc) as tc, ExitStack() as ctx:
        singles = ctx.enter_context(tc.tile_pool(name="singles", bufs=1))
        stg = ctx.enter_context(tc.tile_pool(name="stg", bufs=2))
        hp = ctx.enter_context(tc.tile_pool(name="hp", bufs=2))
        htp = ctx.enter_context(tc.tile_pool(name="htp", bufs=2))
        op = ctx.enter_context(tc.tile_pool(name="op", bufs=2))
        scr = ctx.enter_context(tc.tile_pool(name="scr", bufs=2))
        sm = ctx.enter_context(tc.tile_pool(name="sm", bufs=4))
        pp = ctx.enter_context(tc.tile_pool(name="pp", bufs=2, space="PSUM"))

        wkT_sb = singles.tile([P, d_tiles, DIM], mm_dt)
        wk_view = wkT_sb[:].rearrange("p (s l) e -> p s l e", l=G)

        def load_wk(s):  # loads 2 d-slices at once
            stage = stg.tile([P, G, DIM], fp32, tag="wk_stage")
            nc.sync.dma_start(stage[:], wk_g[s])
            if kscale != 1.0 or dr:
                nc.scalar.activation(
                    wk_view[:, s], stage[:], AF.Copy, bias=0.0, scale=kscale)
            else:
                nc.vector.tensor_copy(wk_view[:, s], stage[:])

        wv_rep = singles.tile([P, DIM], fp32)
        nc.sync.dma_start(wv_rep[:], bcast(wv_d.ap(), DIM))
        wb_rep = singles.tile([P, DIM], fp32)
        nc.sync.dma_start(wb_rep[:], bcast(wb_d.ap(), DIM))
        bb_sb = singles.tile([P, 1], fp32)
        nc.sync.dma_start(bb_sb[:], bcast(bb_d.ap(), 1))
        bv_sb = singles.tile([P, 1], fp32)
        nc.sync.dma_start(bv_sb[:], bcast(bv_d.ap(), 1))
        beta_acc = singles.tile([P, G], fp32)
        nc.vector.memset(beta_acc[:], 0.0)

        gbufs = {}

        def prefetch(g, rep):
            if g >= n_groups or rep >= reps:
                return
            h_f32 = hp.tile([P, G, DIM], fp32, tag="h")
            nc.sync.dma_start(h_f32[:], h_g[g])
            h_bf = hp.tile([P, G, DIM], bf16, tag="hbf")
            nc.scalar.copy(h_bf[:].rearrange("p l d -> p (l d)"),
                           h_f32[:].rearrange("p l d -> p (l d)"))
            hT = htp.tile([P, G, d_tiles, P], bf16, tag="hT")
            nc.sync.dma_start_transpose(
                hT[:].rearrange("p l j t -> p (l j) t"),
                h_bf[:].rearrange("p l d -> p (l d)"))
            if dr:
                hT8 = htp.tile([P, G, d_tiles, P], fp8, tag="hT8")
                nc.vector.tensor_copy(
                    hT8[:].rearrange("p a b c -> p (a b c)"),
                    hT[:].rearrange("p a b c -> p (a b c)"))
                gbufs[(rep, g)] = (h_f32, hT8)
            else:
                gbufs[(rep, g)] = (h_f32, hT)

        # prologue: 2 groups prefetched; wk loads interleaved
        PF = 2
        wk_done = 0
        for g in range(PF):
            prefetch(g, 0)
            target = (d_tiles // G) * (g + 1) // PF
            while wk_done < target:
                load_wk(wk_done)
                wk_done += 1
        while wk_done < d_tiles // G:
            load_wk(wk_done)
            wk_done += 1

        for rep in range(reps):
            for g in range(n_groups):
                h_f32, lht = gbufs.pop((rep, g))
                ssqg = sm.tile([P, G], fp32, tag="ssqg")
                dotg = sm.tile([P, G], fp32, tag="dotg")
                junk3 = scr.tile([P, G, DIM], bf16, tag="jdot")
                kraws = []
                for l in range(G):
                    kraw = pp.tile([P, DIM], fp32, tag="kraw")
                    kraws.append(kraw)
                    if dr:
                        for j in range(d_tiles // 2):
                            lhsT = lht[:, l, 2 * j:2 * j + 2, :]
                            for e in range(e_chunks):
                                nc.tensor.matmul(
                                    kraw[:, e * N_CHUNK:(e + 1) * N_CHUNK],
                                    lhsT,
                                    wkT_sb[:, 2 * j:2 * j + 2,
                                           e * N_CHUNK:(e + 1) * N_CHUNK],
                                    start=(j == 0),
                                    stop=(j == d_tiles // 2 - 1),
                                    perf_mode=perf_mode)
                    else:
                        for j in range(d_tiles):
                            lhsT = lht[:, l, j, :]
                            for e in range(e_chunks):
                                nc.tensor.matmul(
                                    kraw[:, e * N_CHUNK:(e + 1) * N_CHUNK],
                                    lhsT,
                                    wkT_sb[:, j,
                                           e * N_CHUNK:(e + 1) * N_CHUNK],
                                    start=(j == 0), stop=(j == d_tiles - 1))
                    # per-tile consumers that must read PSUM
                    junkS = scr.tile([P, DIM], bf16, tag="jssq")
                    nc.scalar.activation(junkS[:], kraw[:], AF.Square,
                                         accum_out=ssqg[:, l:l + 1])
                    nc.vector.tensor_mul(junk3[:, l, :], kraw[:],
                                         h_f32[:, l, :])

                prefetch(g + PF if g + PF < n_groups else g + PF - n_groups,
                         rep if g + PF < n_groups else rep + 1)

                nc.vector.tensor_reduce(dotg[:], junk3[:], axis=X, op=A.add)

                # v / b batched over the group
                vg = sm.tile([P, G], fp32, tag="vg")
                junk0 = scr.tile([P, G, DIM], bf16, tag="jvb")
                wv_b = bass.AP(tensor=wv_rep.tensor, offset=wv_rep[:].offset,
                               ap=[wv_rep[:].ap[0], [0, G], wv_rep[:].ap[1]])
                nc.gpsimd.tensor_mul(junk0[:], h_f32[:], wv_b)
                nc.vector.tensor_reduce(vg[:], junk0[:], axis=X, op=A.add)
                bg = sm.tile([P, G], fp32, tag="bg")
                junk1 = scr.tile([P, G, DIM], bf16, tag="jvb")
                wb_b = bass.AP(tensor=wb_rep.tensor, offset=wb_rep[:].offset,
                               ap=[wb_rep[:].ap[0], [0, G], wb_rep[:].ap[1]])
                nc.gpsimd.tensor_mul(junk1[:], h_f32[:], wb_b)
                nc.vector.tensor_reduce(bg[:], junk1[:], axis=X, op=A.add)

                # scalar chain on [P, G]
                nrm = sm.tile([P, G], fp32, tag="nrm")
                nc.scalar.activation(nrm[:], ssqg[:], AF.Sqrt)
                nc.vector.tensor_scalar_add(nrm[:], nrm[:], EPS * kscale)
                s = sm.tile([P, G], fp32, tag="s")
                nc.vector.reciprocal(s[:], nrm[:])
                beta = sm.tile([P, G], fp32, tag="beta")
                nc.scalar.activation(beta[:], bg[:], AF.Sigmoid,
                                     bias=bb_sb[:], scale=1.0)
                nc.vector.scalar_tensor_tensor(
                    out=beta_acc[:], in0=beta[:], scalar=2.0, in1=beta_acc[:],
                    op0=A.mult, op1=A.add)
                t1 = sm.tile([P, G], fp32, tag="t1")
                nc.vector.tensor_mul(t1[:], s[:], dotg[:])
                t2 = sm.tile([P, G], fp32, tag="t2")
                nc.vector.scalar_tensor_tensor(
                    out=t2[:], in0=vg[:], scalar=bv_sb[:], in1=t1[:],
                    op0=A.add, op1=A.subtract)
                t3 = sm.tile([P, G], fp32, tag="t3")
                nc.vector.tensor_mul(t3[:], beta[:], s[:])
                alpha = sm.tile([P, G], fp32, tag="alpha")
                nc.vector.scalar_tensor_tensor(
                    out=alpha[:], in0=t3[:], scalar=2.0, in1=t2[:],
                    op0=A.mult, op1=A.mult)

                h_out = op.tile([P, G, DIM], fp32, tag="hout")
                for l in range(G):
                    nc.vector.scalar_tensor_tensor(
                        out=h_out[:, l, :], in0=kraws[l][:],
                        scalar=alpha[:, l:l + 1], in1=h_f32[:, l, :],
                        op0=A.mult, op1=A.add)
                nc.sync.dma_start(out_g[g], h_out[:])

        nc.sync.dma_start(bp_d[:], beta_acc[:])

    nc.compile()
    return nc




def make_in_maps(h, Wk, Wb, bb, Wv, bv, n_cores: int = N_CORES):
    toks = np.asarray(h, dtype=np.float32).reshape(-1, DIM)
    shards = np.split(toks, n_cores, axis=0)
    wkT = np.ascontiguousarray(np.asarray(Wk, dtype=np.float32).T)
    return [
        {"h": np.ascontiguousarray(s), "wkT": wkT,
         "wv": np.asarray(Wv, dtype=np.float32),
         "wb": np.asarray(Wb, dtype=np.float32),
         "bb": np.asarray(bb, dtype=np.float32),
         "bv": np.asarray(bv, dtype=np.float32)}
        for s in shards
    ]


_NC_CACHE = {}


def _get_nc(reps: int = 1, dtype: str = None):
    key = (reps, dtype or DTYPE)
    if key not in _NC_CACHE:
        _NC_CACHE[key] = build(TOKENS_PER_CORE, reps=reps, dtype=dtype)
    return _NC_CACHE[key]


def kernel(h, Wk, Wb, bb, Wv, bv):
    nc = _get_nc()
    in_maps = make_in_maps(h, Wk, Wb, bb, Wv, bv)
    res = run_bass_kernel_spmd(nc, in_maps, list(range(N_CORES)))
    h_next = np.concatenate([r["out"] for r in res.results], axis=0)
    h_next = h_next.reshape(4, 4096, DIM)
    beta_sum = np.sum([r["beta_part"].sum(dtype=np.float64)
                       for r in res.results])
    beta_mean = np.float32(beta_sum / (4 * 4096))
    return h_next, beta_mean
